# revision 1
# baseline (speedup 1.0000x reference)
"""CRF loss kernel for Trainium2 (8 NeuronCores, data-parallel over batch).

Strategy
--------
The loss is mean_b(logZ[b] - real[b]) for a linear-chain CRF with 64 tags
(+2 START/END states), B=512, T=1024.

* logZ (the forward partition function) is the only sequentially-hard part.
  It is computed on-device in exp-space: the log-space forward recursion
      alpha_{t+1}[cur] = obs_t[cur] + LSE_prev(alpha_t[prev] + trans[cur,prev])
  becomes, with A = exp(alpha) (suitably rescaled),
      A_{t+1} = exp(obs_t) * (W^T A_t),   W[prev,cur] = exp(trans[cur,prev] - c)
  i.e. one tiny stationary-weight matmul on PE plus one elementwise multiply
  on DVE per time step.  The constant c (~mean per-step log growth, estimated
  on host) keeps the fp32/bf16 dynamic range centered; the exact correction
  T*c is added back at the end.
* To halve the serial chain length (the wall-clock is latency-bound on the
  PE->PSUM->DVE->SBUF round trip), each core runs TWO independent chains:
  the forward recursion from t=0 and the backward (beta) recursion from
  t=T-1, meeting at t=T/2:
      B_t = W_b (exp(obs_t) * B_{t+1}),   logZ = log(sum_s A_S[s] * B_S[s]) + T*c
* Batch is sharded 512 -> 8 x 64; each core's emission slab is pre-arranged
  on host to [step, 66, 128] (cols 0:64 forward step s, cols 64:128 backward
  step T-1-s, rows 64:65 zero so exp gives 1.0 for START/END emissions),
  streamed in chunks and exponentiated on ACT.
* The "real path" score (emission gather + transition lookups along the
  given tag sequence) is a trivially-parallel gather; it is computed on host
  in vectorized numpy, as is the final scalar mean (the all-reduce).

The kernel assumes mask is all ones (the problem spec fills it with ones).
"""

import numpy as np
import ml_dtypes
from contextlib import ExitStack

import concourse.bass as bass
import concourse.tile as tile
from concourse import bacc, mybir
from concourse.bass_utils import run_bass_kernel_spmd

TAG = 64
NE = 66
START = 64
END = 65
B = 512
T = 1024
S = T // 2          # steps per chain (fwd + bwd meet in the middle)
NCORES = 8
BC = B // NCORES    # batch per core
CH = 32             # time steps per DMA/exp chunk
NCH = S // CH

BF16 = ml_dtypes.bfloat16

_PROGRAM_CACHE = {}


def _build_program():
    nc = bacc.Bacc(
        "TRN2", target_bir_lowering=False, debug=False, num_devices=NCORES
    )
    f32 = mybir.dt.float32
    bf16 = mybir.dt.bfloat16

    slab = nc.dram_tensor("slab", [S, NE, 2 * BC], f32, kind="ExternalInput").ap()
    wf = nc.dram_tensor("wf", [NE, NE], bf16, kind="ExternalInput").ap()
    wb = nc.dram_tensor("wb", [NE, NE], bf16, kind="ExternalInput").ap()
    a0 = nc.dram_tensor("a0", [NE, BC], bf16, kind="ExternalInput").ap()
    b0 = nc.dram_tensor("b0", [NE, BC], f32, kind="ExternalInput").ap()
    ones = nc.dram_tensor("ones", [NE, 1], f32, kind="ExternalInput").ap()
    out = nc.dram_tensor("norm_out", [1, BC], f32, kind="ExternalOutput").ap()

    with tile.TileContext(nc) as tc, ExitStack() as ctx:
        consts = ctx.enter_context(tc.tile_pool(name="consts", bufs=1))
        raws = ctx.enter_context(tc.tile_pool(name="raws", bufs=2))
        dpool = ctx.enter_context(tc.tile_pool(name="dpool", bufs=2))
        stf = ctx.enter_context(tc.tile_pool(name="stf", bufs=3))
        stb = ctx.enter_context(tc.tile_pool(name="stb", bufs=3))
        ppsa = ctx.enter_context(tc.tile_pool(name="ppsa", bufs=2, space="PSUM"))
        ppsb = ctx.enter_context(tc.tile_pool(name="ppsb", bufs=2, space="PSUM"))
        ppsv = ctx.enter_context(tc.tile_pool(name="ppsv", bufs=1, space="PSUM"))

        wf_t = consts.tile([NE, NE], bf16)
        nc.sync.dma_start(out=wf_t, in_=wf)
        wb_t = consts.tile([NE, NE], bf16)
        nc.sync.dma_start(out=wb_t, in_=wb)
        a_cur = consts.tile([NE, BC], bf16)
        nc.sync.dma_start(out=a_cur, in_=a0)
        b_init = consts.tile([NE, BC], f32)
        nc.sync.dma_start(out=b_init, in_=b0)
        ones_t = consts.tile([NE, 1], f32)
        nc.sync.dma_start(out=ones_t, in_=ones)

        slab_r = slab.rearrange("t p c -> p t c")  # [66, S, 128]
        ps_b = None
        for ch in range(NCH):
            raw = raws.tile([NE, CH, 2 * BC], f32)
            nc.sync.dma_start(
                out=raw, in_=slab_r[:, ch * CH : (ch + 1) * CH, :]
            )
            d = dpool.tile([NE, CH, 2 * BC], f32)
            nc.scalar.activation(d, raw, mybir.ActivationFunctionType.Exp)
            for j in range(CH):
                # forward chain: matmul then elementwise multiply
                ps_a = ppsa.tile([NE, BC], f32)
                nc.tensor.matmul(ps_a, wf_t, a_cur, start=True, stop=True)
                a_new = stf.tile([NE, BC], bf16)
                nc.vector.tensor_mul(a_new, ps_a, d[:, j, 0:BC])
                a_cur = a_new
                # backward chain: elementwise multiply then matmul
                bm = stb.tile([NE, BC], bf16)
                nc.vector.tensor_mul(
                    bm, ps_b if ps_b is not None else b_init, d[:, j, BC : 2 * BC]
                )
                ps_b2 = ppsb.tile([NE, BC], f32)
                nc.tensor.matmul(ps_b2, wb_t, bm, start=True, stop=True)
                ps_b = ps_b2

        # seam: logZ contribution = log(sum_state A_S * B_S)
        p_t = stf.tile([NE, BC], f32, tag="seam")
        nc.vector.tensor_mul(p_t, ps_b, a_cur)
        ps_v = ppsv.tile([1, BC], f32)
        nc.tensor.matmul(ps_v, ones_t, p_t, start=True, stop=True)
        ln_t = stf.tile([1, BC], f32, tag="lnout")
        nc.scalar.activation(ln_t, ps_v, mybir.ActivationFunctionType.Ln)
        nc.sync.dma_start(out=out, in_=ln_t)

    nc.compile()
    return nc


def _get_program():
    if "nc" not in _PROGRAM_CACHE:
        _PROGRAM_CACHE["nc"] = _build_program()
    return _PROGRAM_CACHE["nc"]


def _estimate_c(logits, transitions, nb=16, nt=64, skip=8):
    """Mean per-step log growth of the forward DP (host, small sample)."""
    NEG = -10000.0
    lg = np.concatenate(
        [logits[:nb, :nt], np.zeros((nb, nt, 2), np.float32)], axis=-1
    ).astype(np.float64)
    tr = transitions.astype(np.float64)
    prevs = np.full((nb, NE), NEG)
    prevs[:, START] = 0.0

    def lse(x, ax):
        m = x.max(axis=ax, keepdims=True)
        return (m + np.log(np.exp(x - m).sum(axis=ax, keepdims=True))).squeeze(ax)

    growths = []
    tot_prev = lse(prevs, 1)
    for t in range(nt):
        scores = prevs[:, None, :] + lg[:, t, :, None] + tr[None, :, :]
        prevs = lse(scores, 2)
        tot = lse(prevs, 1)
        growths.append((tot - tot_prev).mean())
        tot_prev = tot
    return float(np.mean(growths[skip:]))


def _real_path_score(logits, mask, tags, transitions):
    """Vectorized host computation of the labeled-path score. [B]"""
    lg = np.concatenate([logits, np.zeros((B, T, 2), logits.dtype)], axis=-1)
    maskf = mask.astype(np.float64)
    tags_m = np.where(mask, tags, END).astype(np.int64)
    emis = np.take_along_axis(lg, tags_m[:, :, None], axis=2)[..., 0].astype(
        np.float64
    )
    emis = (emis * maskf).sum(axis=1)
    tags_ext = np.concatenate(
        [
            np.full((B, 1), START, np.int64),
            tags_m,
            np.full((B, 1), END, np.int64),
        ],
        axis=1,
    )
    trn = transitions.astype(np.float64)[tags_ext[:, 1:], tags_ext[:, :-1]]
    mask_ext = np.concatenate([np.ones((B, 1), np.float64), maskf], axis=1)
    return emis + (trn * mask_ext).sum(axis=1)


def _make_inputs(logits, transitions, c):
    """Per-core input maps for the device program."""
    tr = transitions.astype(np.float32)
    wf_np = np.exp(tr - c).T.astype(BF16)  # lhsT fwd: [prev, cur]
    wb_np = np.exp(tr - c).astype(BF16)   # lhsT bwd: [cur, prev]
    a0_np = np.zeros((NE, BC), BF16)
    a0_np[START, :] = 1.0
    b0_np = np.repeat(np.exp(tr[END])[:, None], BC, axis=1).astype(np.float32)
    ones_np = np.ones((NE, 1), np.float32)

    in_maps = []
    for k in range(NCORES):
        bs = slice(k * BC, (k + 1) * BC)
        lgk = logits[bs]  # [BC, T, TAG]
        lgt = np.ascontiguousarray(lgk.transpose(1, 2, 0))  # [T, TAG, BC]
        slab = np.zeros((S, NE, 2 * BC), np.float32)
        slab[:, 0:TAG, 0:BC] = lgt[0:S]
        slab[:, 0:TAG, BC : 2 * BC] = lgt[S:T][::-1]
        in_maps.append(
            {
                "slab": slab,
                "wf": wf_np,
                "wb": wb_np,
                "a0": a0_np,
                "b0": b0_np,
                "ones": ones_np,
            }
        )
    return in_maps


def _run(logits, mask, tags, transitions, trace=False, **spmd_kwargs):
    logits = np.asarray(logits, dtype=np.float32)
    mask = np.asarray(mask).astype(bool)
    tags = np.asarray(tags).astype(np.int64)
    transitions = np.asarray(transitions, dtype=np.float32)

    c = _estimate_c(logits, transitions)
    real = _real_path_score(logits, mask, tags, transitions)

    nc = _get_program()
    in_maps = _make_inputs(logits, transitions, c)
    res = run_bass_kernel_spmd(
        nc, in_maps, list(range(NCORES)), trace=trace, **spmd_kwargs
    )
    norms = np.concatenate(
        [res.results[k]["norm_out"].reshape(BC) for k in range(NCORES)]
    ).astype(np.float64)
    norms = norms + T * c
    loss = (norms - real).mean()
    return np.float32(loss), res


def kernel(logits, mask, tags, transitions):
    loss, _ = _run(logits, mask, tags, transitions, trace=False)
    return np.array(loss, dtype=np.float32)
